# revision 17
# baseline (speedup 1.0000x reference)
"""Trainium2 Bass kernel: minGRU block (emb-gather -> rmsnorm -> minGRU scan ->
MLP -> rmsnorm -> logits + log_softmax loss) on 8 NeuronCores.

Sharding: trunk token-sharded (8 chunks of 512 tokens; the reference's
log-space scan is reproduced faithfully -- including XLA-CPU exp
flush-to-zero -- with chunked prefix state exchanged via small AllGathers);
logits vocab-sharded (4000 cols/core) after an AllGather of the normalized
activations. Collectives are channel-split so their latency hides behind
the other half's compute. Host does gather/shard/concat + tiny loss combine.
"""
import os
import sys

import numpy as np
import ml_dtypes

for _p in ("/opt/trn_rl_repo", "/root/.axon_site/_ro/trn_rl_repo"):
    if os.path.isdir(_p) and _p not in sys.path:
        sys.path.append(_p)

import concourse.bass as bass
import concourse.tile as tile
import concourse.mybir as mybir
from concourse.bass_utils import run_bass_kernel_spmd
from concourse.masks import make_identity
from concourse.vector_clock import ScopedClock

dt = mybir.dt
AF = mybir.ActivationFunctionType
OP = mybir.AluOpType
AX = mybir.AxisListType
bf16 = ml_dtypes.bfloat16

NCORES = 8
B, S, D, V, F = 2, 2048, 512, 32000, 2048
T = 512              # tokens per core chunk
NT = D // 128        # 4 channel tiles
VS = V // NCORES     # 4000 vocab cols per core
SL = 500             # vocab slice per psum tile
NSL = VS // SL       # 8
MT = (B * S) // 128  # 32 token m-tiles
NTOK = B * S

EXP_FLUSH = -87.3365445          # XLA-CPU exp flush-to-zero cutoff
LN_SQRT_D = float(np.log(np.sqrt(np.float64(D))))
ERF_P = 0.47047                  # Abramowitz-Stegun 7.1.25
ERF_A = (0.3480242, -0.0958798, 0.7478556)
SQ2I = float(1.0 / np.sqrt(2.0))
LN2_64 = float(64 * np.log(2.0))

# ---------------------------------------------------------------------------
# Workaround: this container's walrus rejects instructions carrying more than
# one semaphore wait. Split extra waits onto single-wait NoOps (same engine).
# ---------------------------------------------------------------------------
_split_ctr = [0]


def _patched_drain_and_barrier(self, tick_clock, wait_clock):
    probe = self.nc.sync.nop(nofuse=True, hint="drain_waits")
    wait_clock.add_sem_waits(probe.ins, ScopedClock({None: tick_clock.global_clock}))
    si = probe.ins.sync_info
    if si is not None and si.on_wait and len(si.on_wait) > 1:
        extra = list(si.on_wait[1:])
        del si.on_wait[1:]
        for w in extra:
            n2 = self.nc.sync.nop(nofuse=True, hint="drain_waits")
            n2.ins.sync_info = mybir.SyncInfo(on_wait=[w], on_update=[])
    self.nc.sync.drain()
    self.nc.all_engine_barrier()
    assert self.sems is not None
    popped = self.nc._tile_sem_poison_stack.pop()
    assert popped is self._sem_poison
    self.nc.clear_and_free_semaphores(list(self.sems.allocated().values()))
    self.nc.all_engine_barrier()


tile.TileContext._drain_and_barrier = _patched_drain_and_barrier


def _split_multi_waits(nc):
    for fn in nc.m.functions:
        for bb in fn.blocks:
            new_insts = []
            for inst in bb.instructions:
                si = inst.sync_info
                if si is not None and si.on_wait and len(si.on_wait) > 1:
                    extra = list(si.on_wait[:-1])
                    keep = si.on_wait[-1]
                    for w in extra:
                        _split_ctr[0] += 1
                        new_insts.append(mybir.InstNoOp(
                            name=f"I-swsplit-{_split_ctr[0]}",
                            engine=inst.engine,
                            sync_info=mybir.SyncInfo(on_wait=[w], on_update=[]),
                            text_hint="split_wait",
                            bass_nofuse=True,
                        ))
                    si.on_wait = [keep]
                new_insts.append(inst)
            bb.instructions[:] = new_insts


# ---------------------------------------------------------------------------
# Device program
# ---------------------------------------------------------------------------
def _build():
    nc = bass.Bass("TRN2", target_bir_lowering=False, debug=False,
                   num_devices=NCORES)
    f32 = dt.float32

    x0_in = nc.dram_tensor("x0", [T, D], f32, kind="ExternalInput").ap()
    g1_in = nc.dram_tensor("g1", [D], f32, kind="ExternalInput").ap()
    g2_in = nc.dram_tensor("g2", [D], f32, kind="ExternalInput").ap()
    wg_in = nc.dram_tensor("wg", [D, 2 * D], f32, kind="ExternalInput").ap()
    w1_in = nc.dram_tensor("w1", [D, F], dt.float32r, kind="ExternalInput").ap()
    b1_in = nc.dram_tensor("b1", [F], f32, kind="ExternalInput").ap()
    w2_in = nc.dram_tensor("w2", [F, D], dt.bfloat16, kind="ExternalInput").ap()
    b2_in = nc.dram_tensor("b2", [D], f32, kind="ExternalInput").ap()
    wlg_in = nc.dram_tensor("wlg", [D, VS], dt.bfloat16, kind="ExternalInput").ap()
    wlab_in = nc.dram_tensor("wlab", [D, T], dt.bfloat16, kind="ExternalInput").ap()
    mask_in = nc.dram_tensor("mask", [128, 8], f32, kind="ExternalInput").ap()
    selb_in = nc.dram_tensor("selb", [128, 1], f32, kind="ExternalInput").ap()

    logits_out = nc.dram_tensor("logits", [NTOK, VS], f32, kind="ExternalOutput").ap()
    labv_out = nc.dram_tensor("labv", [1, T], f32, kind="ExternalOutput").ap()
    sexp_out = nc.dram_tensor("sexp", [128, MT], f32, kind="ExternalOutput").ap()
    nh_out = nc.dram_tensor("nh", [2, D], f32, kind="ExternalOutput").ap()

    with tile.TileContext(nc) as tc:
        with tc.tile_pool(name="const", bufs=1) as cp, \
             tc.tile_pool(name="io", bufs=1) as io, \
             tc.tile_pool(name="big", bufs=4) as bigp, \
             tc.tile_pool(name="work", bufs=4) as wp, \
             tc.tile_pool(name="ps", bufs=8, space="PSUM") as pp, \
             tc.tile_pool(name="dram", bufs=1, space="DRAM") as dp:

            # ---- warm-up collective: absorbs cross-core launch skew while
            # input DMAs stream (first collective ~60us, later ones ~13us) ----
            warm_b = dp.tile([1, 128], f32)
            warm_g = dp.tile([NCORES, 128], f32, addr_space="Shared")
            nc.gpsimd.collective_compute(
                "AllGather", OP.bypass, replica_groups=[list(range(NCORES))],
                ins=[warm_b.opt()], outs=[warm_g.opt()])

            # ---- constants ----
            ident = cp.tile([128, 128], f32, tag="ident")
            make_identity(nc, ident[:])
            zeros = cp.tile([128, 512], f32, tag="zeros")
            nc.vector.memset(zeros[:], 0.0)
            ones128 = cp.tile([128, 1], f32, tag="ones128")
            nc.vector.memset(ones128[:], 1.0)
            ones1x = cp.tile([1, 128], f32, tag="ones1x")
            nc.vector.memset(ones1x[:], 1.0)
            onesbf = cp.tile([128, 1], dt.bfloat16, tag="onesbf")
            nc.vector.memset(onesbf[:], 1.0)
            lnsqd = cp.tile([128, 1], f32, tag="lnsqd")
            nc.vector.memset(lnsqd[:], LN_SQRT_D)

            g1p = cp.tile([128, NT], f32, tag="g1p")
            nc.sync.dma_start(out=g1p[:], in_=g1_in.rearrange("(c p) -> p c", p=128))
            nc.vector.tensor_scalar_add(g1p[:], g1p[:], 1.0)
            g2p = cp.tile([128, NT], f32, tag="g2p")
            nc.sync.dma_start(out=g2p[:], in_=g2_in.rearrange("(c p) -> p c", p=128))
            nc.vector.tensor_scalar_add(g2p[:], g2p[:], 1.0)
            b1t = cp.tile([128, F // 128], f32, tag="b1t")
            nc.sync.dma_start(out=b1t[:], in_=b1_in.rearrange("(m p) -> p m", p=128))
            b1s = cp.tile([128, F // 128], f32, tag="b1s")
            nc.vector.tensor_scalar_mul(b1s[:], b1t[:], SQ2I)
            b2t = cp.tile([128, NT], f32, tag="b2t")
            nc.sync.dma_start(out=b2t[:], in_=b2_in.rearrange("(c p) -> p c", p=128))
            maskt = cp.tile([128, 8], f32, tag="maskt")
            nc.sync.dma_start(out=maskt[:], in_=mask_in[:])
            selb = cp.tile([128, 1], f32, tag="selb")
            nc.sync.dma_start(out=selb[:], in_=selb_in[:])
            sexp_all = cp.tile([128, MT], f32, tag="sexp_all")

            _scr_n = [0]

            def scr(tag="scr", shape=(128, 512), dtype=f32, bufs=7):
                _scr_n[0] += 1
                return wp.tile(list(shape), dtype, tag=tag, bufs=bufs,
                               name=f"{tag}_{_scr_n[0]}")

            # ---- stage 1: rms1 + transpose -> xhT (feature-major) ----
            xh0 = []
            for t in range(NT):
                x0t = scr("x0t", bufs=2)
                nc.sync.dma_start(out=x0t[:], in_=x0_in[t * 128:(t + 1) * 128, :])
                ssq = scr("t1", (128, 1), bufs=16)
                sq_s = scr()
                nc.scalar.activation(sq_s[:], x0t[:], AF.Square, accum_out=ssq[:])
                lssq = scr("t1", (128, 1), bufs=16)
                nc.scalar.activation(lssq[:], ssq[:], AF.Ln)
                r = scr("t1", (128, 1), bufs=16)
                nc.scalar.activation(r[:], lssq[:], AF.Exp, scale=-0.5,
                                     bias=lnsqd[:, 0:1])
                xh = scr("xh0", bufs=4)
                nc.vector.tensor_scalar_mul(xh[:], x0t[:], r[:, 0:1])
                xh0.append(xh)

            wgt = [io.tile([128, 2 * D], f32, tag=f"wgt{k}", name=f"wgt{k}")
                   for k in range(NT)]
            for k in range(NT):
                nc.sync.dma_start(out=wgt[k][:], in_=wg_in[k * 128:(k + 1) * 128, :])

            xhT = []
            for c in range(NT):
                psx = pp.tile([128, 512], f32, tag="pp")
                for t in range(NT):
                    nc.tensor.transpose(psx[:, t * 128:(t + 1) * 128],
                                        xh0[t][:, c * 128:(c + 1) * 128], ident[:])
                xt_ = wp.tile([128, T], f32, tag="xhT")
                nc.vector.tensor_scalar_mul(xt_[:], psx[:], g1p[:, c:c + 1])
                xhT.append(xt_)

            # ---- remaining weight loads (stream during trunk compute) ----
            w1t = [bigp.tile([128, F], dt.float32r, tag="w1y", name=f"w1t{_k}")
                   for _k in range(NT)]
            for k in range(NT):
                nc.sync.dma_start(out=w1t[k][:], in_=w1_in[k * 128:(k + 1) * 128, :])
            w2t = [io.tile([128, D], dt.bfloat16, tag=f"w2t{k}", name=f"w2t{k}")
                   for k in range(F // 128)]
            for k in range(F // 128):
                nc.sync.dma_start(out=w2t[k][:], in_=w2_in[k * 128:(k + 1) * 128, :])
            wlabt = [io.tile([128, T], dt.bfloat16, tag=f"wlab{c}", name=f"wlab{c}")
                     for c in range(NT)]
            for c in range(NT):
                nc.sync.dma_start(out=wlabt[c][:], in_=wlab_in[c * 128:(c + 1) * 128, :])
            wlgt = [io.tile([128, VS], dt.bfloat16, tag=f"wlg{k}", name=f"wlg{k}")
                    for k in range(NT)]
            for k in range(NT):
                nc.sync.dma_start(out=wlgt[k][:], in_=wlg_in[k * 128:(k + 1) * 128, :])

            # ---- stage 2: hg matmuls (fp32) + local log-space scan ----
            las = [None] * NT
            nloc = [None] * NT
            mnl_l = [None] * NT
            Sl = [None] * NT
            cumA_l = [None] * NT
            obias_l = [None] * NT
            ubias_l = [None] * NT
            M2b_l = [None] * NT
            x1 = [None] * NT

            def phaseA(c):
                psh = pp.tile([128, T], f32, tag="pp", name=f"psh{c}")
                for k in range(NT):
                    nc.tensor.matmul(psh[:], wgt[k][:, c * 128:(c + 1) * 128],
                                     xhT[k][:], start=(k == 0), stop=(k == NT - 1))
                psg = pp.tile([128, T], f32, tag="pp", name=f"psg{c}")
                for k in range(NT):
                    nc.tensor.matmul(psg[:], wgt[k][:, (NT + c) * 128:(NT + c + 1) * 128],
                                     xhT[k][:], start=(k == 0), stop=(k == NT - 1))
                # gate: lc = -softplus(gate); lz = ln(1+e^-gate) (= -log_z)
                epg = scr()
                nc.scalar.activation(epg[:], psg[:], AF.Exp)
                nc.vector.tensor_scalar_add(epg[:], epg[:], 1.0)
                splus = scr()
                nc.scalar.activation(splus[:], epg[:], AF.Ln)
                lc = scr("lc", bufs=2)
                nc.vector.tensor_scalar_mul(lc[:], splus[:], -1.0)
                eng = scr()
                nc.scalar.activation(eng[:], psg[:], AF.Exp, scale=-1.0)
                nc.vector.tensor_scalar_add(eng[:], eng[:], 1.0)
                lz = scr()
                nc.scalar.activation(lz[:], eng[:], AF.Ln)
                # hidden: lg = h>=0 ? ln(h+0.5) : -ln(1+e^-h)
                hp5 = scr()
                nc.vector.tensor_scalar(hp5[:], psh[:], 0.5, 0.5, OP.add, OP.max)
                lgb1 = scr()
                nc.scalar.activation(lgb1[:], hp5[:], AF.Ln)
                enh = scr()
                nc.scalar.activation(enh[:], psh[:], AF.Exp, scale=-1.0)
                nc.vector.tensor_scalar_add(enh[:], enh[:], 1.0)
                lgb2 = scr()
                nc.scalar.activation(lgb2[:], enh[:], AF.Ln)
                nc.vector.tensor_scalar_mul(lgb2[:], lgb2[:], -1.0)
                mneg = scr("mneg", (128, 512), dt.uint8, bufs=2)
                nc.vector.tensor_scalar(mneg[:], psh[:], 0.0, None, OP.is_lt)
                nc.vector.copy_predicated(lgb1[:], mneg[:], lgb2[:])
                lv = scr()
                nc.vector.tensor_sub(lv[:], lgb1[:], lz[:])
                la = wp.tile([128, T], f32, tag="las", name=f"las{c}")
                nc.vector.tensor_tensor_scan(la[:], lc[:], zeros[:], 0.0,
                                             OP.add, OP.add)
                nl = wp.tile([128, T], f32, tag="nloc", name=f"nloc{c}")
                nc.vector.tensor_sub(nl[:], la[:], lv[:])
                mn = scr("t1", (128, 1), bufs=16)
                nc.vector.tensor_reduce(mn[:], nl[:], AX.X, OP.min)
                las[c] = la; nloc[c] = nl; mnl_l[c] = mn

            def coll1(h, cs):
                w_ = 128 * len(cs)
                sm = dp.tile([2, w_], f32, name=f"sm1_{h}")
                for i, c in enumerate(cs):
                    nc.sync.dma_start(
                        out=sm[0:1, i * 128:(i + 1) * 128].rearrange("o p -> p o"),
                        in_=las[c][:, T - 1:T])
                    nc.sync.dma_start(
                        out=sm[1:2, i * 128:(i + 1) * 128].rearrange("o p -> p o"),
                        in_=mnl_l[c][:, 0:1])
                gth = dp.tile([2 * NCORES, w_], f32, addr_space="Shared",
                              name=f"gth1_{h}")
                nc.gpsimd.collective_compute(
                    "AllGather", OP.bypass, replica_groups=[list(range(NCORES))],
                    ins=[sm.opt()], outs=[gth.opt()])
                return gth[:, :].rearrange("(j r) (c p) -> r c p j", r=2, c=len(cs))

            def post1(c, g1v, i):
                totAg = scr("t8", (128, 8), bufs=16)
                nc.sync.dma_start(out=totAg[:], in_=g1v[0, i])
                mng = scr("t8", (128, 8), bufs=16)
                nc.sync.dma_start(out=mng[:], in_=g1v[1, i])
                cumA = wp.tile([128, 8], f32, tag="cAi", name=f"cAi{c}")
                for b in range(2):
                    nc.vector.tensor_tensor_scan(
                        cumA[:, b * 4:(b + 1) * 4], totAg[:, b * 4:(b + 1) * 4],
                        zeros[:, 0:4], 0.0, OP.add, OP.add)
                cumAx = scr("t8", (128, 8), bufs=16)
                nc.vector.tensor_sub(cumAx[:], cumA[:], totAg[:])
                ng = scr("t8", (128, 8), bufs=16)
                nc.vector.tensor_add(ng[:], mng[:], cumAx[:])
                mnb0 = wp.tile([128, 1], f32, tag="mnb0", name=f"mnb0_{c}")
                nc.vector.tensor_reduce(mnb0[:], ng[:, 0:4], AX.X, OP.min)
                mnb1 = wp.tile([128, 1], f32, tag="mnb1", name=f"mnb1_{c}")
                nc.vector.tensor_reduce(mnb1[:], ng[:, 4:8], AX.X, OP.min)
                dmn = scr("t1", (128, 1), bufs=16)
                nc.vector.tensor_sub(dmn[:], mnb0[:], mnb1[:])
                M2 = scr("t1", (128, 1), bufs=16)
                nc.vector.scalar_tensor_tensor(M2[:], dmn[:], selb[:, 0:1],
                                               mnb1[:], OP.mult, OP.add)
                tm8 = scr("t8", (128, 8), bufs=16)
                nc.vector.tensor_mul(tm8[:], cumA[:], maskt[:])
                Aprev = scr("t1", (128, 1), bufs=16)
                nc.vector.tensor_reduce(Aprev[:], tm8[:], AX.X, OP.add)
                ub = wp.tile([128, 1], f32, tag="ubias", name=f"ub{c}")
                nc.vector.tensor_sub(ub[:], M2[:], Aprev[:])
                ob = wp.tile([128, 1], f32, tag="obias", name=f"ob{c}")
                nc.vector.tensor_sub(ob[:], Aprev[:], M2[:])
                cumA_l[c] = cumA; obias_l[c] = ob
                ubias_l[c] = ub; M2b_l[c] = (mnb0, mnb1)

            def phaseB(c):
                arg = scr()
                nc.vector.tensor_scalar(arg[:], nloc[c][:], -1.0,
                                        ubias_l[c][:, 0:1], OP.mult, OP.add)
                u = scr()
                nc.scalar.activation(u[:], arg[:], AF.Exp)
                um = scr("mneg", (128, 512), dt.uint8, bufs=2)
                nc.vector.tensor_scalar(um[:], arg[:], EXP_FLUSH, None, OP.is_lt)
                nc.vector.copy_predicated(u[:], um[:], zeros[:])
                Sc = wp.tile([128, T], f32, tag="Sl", name=f"Sl{c}")
                nc.vector.tensor_tensor_scan(Sc[:], u[:], zeros[:], 0.0,
                                             OP.add, OP.add)
                Sl[c] = Sc

            def coll2(h, cs):
                w_ = 128 * len(cs)
                sm = dp.tile([1, w_], f32, name=f"sm2_{h}")
                for i, c in enumerate(cs):
                    nc.sync.dma_start(
                        out=sm[0:1, i * 128:(i + 1) * 128].rearrange("o p -> p o"),
                        in_=Sl[c][:, T - 1:T])
                gth = dp.tile([NCORES, w_], f32, addr_space="Shared",
                              name=f"gth2_{h}")
                nc.gpsimd.collective_compute(
                    "AllGather", OP.bypass, replica_groups=[list(range(NCORES))],
                    ins=[sm.opt()], outs=[gth.opt()])
                return gth[:, :].rearrange("j (c p) -> c p j", c=len(cs))

            def phaseC(c, g2v, i):
                usg = scr("t8", (128, 8), bufs=16)
                nc.sync.dma_start(out=usg[:], in_=g2v[i])
                cumU = scr("t8", (128, 8), bufs=16)
                for b in range(2):
                    nc.vector.tensor_tensor_scan(
                        cumU[:, b * 4:(b + 1) * 4], usg[:, b * 4:(b + 1) * 4],
                        zeros[:, 0:4], 0.0, OP.add, OP.add)
                cumUx = scr("t8", (128, 8), bufs=16)
                nc.vector.tensor_sub(cumUx[:], cumU[:], usg[:])
                tm8 = scr("t8", (128, 8), bufs=16)
                nc.vector.tensor_mul(tm8[:], cumUx[:], maskt[:])
                Sprev = scr("t1", (128, 1), bufs=16)
                nc.vector.tensor_reduce(Sprev[:], tm8[:], AX.X, OP.add)
                Sfull = scr()
                nc.vector.tensor_scalar_add(Sfull[:], Sl[c][:], Sprev[:, 0:1])
                lnS = scr()
                nc.scalar.activation(lnS[:], Sfull[:], AF.Ln)
                # ACT Ln clamps below ~2e-19: rescale small S by 2^64
                Ss = scr()
                nc.vector.tensor_scalar_mul(Ss[:], Sfull[:], float(2.0 ** 64))
                lnB = scr()
                nc.scalar.activation(lnB[:], Ss[:], AF.Ln)
                nc.vector.tensor_scalar_add(lnB[:], lnB[:], -LN2_64)
                msm = scr("mneg", (128, 512), dt.uint8, bufs=2)
                nc.vector.tensor_scalar(msm[:], Sfull[:], 1e-15, None, OP.is_lt)
                nc.vector.copy_predicated(lnS[:], msm[:], lnB[:])
                zm = scr("mneg", (128, 512), dt.uint8, bufs=2)
                nc.vector.tensor_scalar(zm[:], Sfull[:], 0.0, None, OP.is_equal)
                ex = scr()
                nc.vector.tensor_add(ex[:], las[c][:], lnS[:])
                nc.vector.tensor_scalar_add(ex[:], ex[:], obias_l[c][:, 0:1])
                gru = scr()
                nc.scalar.activation(gru[:], ex[:], AF.Exp)
                nc.vector.copy_predicated(gru[:], zm[:], zeros[:])
                xx = wp.tile([128, T], f32, tag="las", name=f"x1_{c}")
                nc.vector.tensor_add(xx[:], gru[:], xhT[c][:])
                x1[c] = xx
                # next_hidden from gathered summaries (cols 3 and 7)
                for bi, col in ((0, 3), (1, 7)):
                    cu3 = scr("t1", (128, 1), bufs=16)
                    nc.vector.tensor_copy(cu3[:], cumU[:, col:col + 1])
                    lnu = scr("t1", (128, 1), bufs=16)
                    nc.scalar.activation(lnu[:], cu3[:], AF.Ln)
                    t6 = scr("t1", (128, 1), bufs=16)
                    nc.vector.tensor_add(t6[:], cumA_l[c][:, col:col + 1], lnu[:])
                    nc.vector.tensor_sub(t6[:], t6[:], M2b_l[c][bi][:])
                    nc.vector.tensor_scalar_max(t6[:], t6[:], -100.0)
                    nhv = scr("t1", (128, 1), bufs=16)
                    nc.scalar.activation(nhv[:], t6[:], AF.Exp)
                    zm1 = scr("zm1", (128, 1), dt.uint8, bufs=4)
                    nc.vector.tensor_scalar(zm1[:], cu3[:], 0.0, None, OP.is_equal)
                    nc.vector.copy_predicated(nhv[:], zm1[:], zeros[:, 0:1])
                    nc.sync.dma_start(
                        out=nh_out[bi:bi + 1, c * 128:(c + 1) * 128].rearrange(
                            "o p -> p o"),
                        in_=nhv[:, 0:1])

            # channel-split pipeline: each collective's latency hides behind
            # the other half's compute
            H0, H1 = [0, 1], [2, 3]
            ALL4 = [0, 1, 2, 3]
            phaseA(0); phaseA(1); phaseA(2); phaseA(3)
            g1 = coll1(0, ALL4)
            for c in ALL4:
                post1(c, g1, c)
                phaseB(c)
            g2 = coll2(0, ALL4)
            for c in ALL4:
                phaseC(c, g2, c)

            # ---- stage 3: MLP (w1 f32r, gelu via erf poly, w2 bf16) ----
            x1r = []
            for c in range(NT):
                xr = wp.tile([128, T], dt.float32r, tag="nloc", name=f"x1r{c}")
                nc.vector.tensor_copy(xr[:], x1[c][:])
                x1r.append(xr)

            h1 = []
            for m in range(F // 128):
                ps1 = pp.tile([128, T], f32, tag="pp", name=f"ps1_{m}")
                for k in range(NT):
                    nc.tensor.matmul(ps1[:], w1t[k][:, m * 128:(m + 1) * 128],
                                     x1r[k][:], start=(k == 0), stop=(k == NT - 1))
                # gelu = 0.5*xg + 0.5*|xg|*(1-q), q = erf-poly(|xg|/sqrt2)*e^-t^2
                tt = scr()
                nc.scalar.activation(tt[:], ps1[:], AF.Abs, scale=SQ2I,
                                     bias=b1s[:, m:m + 1])
                ax = scr()
                nc.scalar.activation(ax[:], ps1[:], AF.Abs, bias=b1t[:, m:m + 1])
                wv = scr()
                nc.vector.tensor_scalar(wv[:], tt[:], ERF_P, 1.0, OP.mult, OP.add)
                lnw = scr()
                nc.scalar.activation(lnw[:], wv[:], AF.Ln)
                kv = scr()
                nc.scalar.activation(kv[:], lnw[:], AF.Exp, scale=-1.0)
                a1 = scr()
                nc.vector.tensor_scalar_mul(a1[:], kv[:], ERF_A[2])
                a2 = scr()
                nc.vector.scalar_tensor_tensor(a2[:], a1[:], ERF_A[1], kv[:],
                                               OP.add, OP.mult)
                a3 = scr()
                nc.vector.scalar_tensor_tensor(a3[:], a2[:], ERF_A[0], kv[:],
                                               OP.add, OP.mult)
                sq2 = scr()
                nc.scalar.activation(sq2[:], tt[:], AF.Square)
                ee = scr()
                nc.scalar.activation(ee[:], sq2[:], AF.Exp, scale=-1.0)
                qv = scr()
                nc.vector.tensor_mul(qv[:], a3[:], ee[:])
                nq = scr()
                nc.vector.tensor_scalar(nq[:], qv[:], -0.5, 0.5, OP.mult, OP.add)
                r3 = scr()
                nc.vector.tensor_mul(r3[:], ax[:], nq[:])
                hx = scr()
                nc.vector.tensor_scalar(hx[:], ps1[:], b1t[:, m:m + 1], 0.5,
                                        OP.add, OP.mult)
                hm = wp.tile([128, T], dt.bfloat16, tag="h1", bufs=16,
                             name=f"h1_{m}")
                nc.vector.tensor_add(hm[:], r3[:], hx[:])
                h1.append(hm)

            x2 = []
            for c in range(NT):
                ps2 = pp.tile([128, T], f32, tag="pp", name=f"ps2_{c}")
                for k in range(F // 128):
                    nc.tensor.matmul(ps2[:], w2t[k][:, c * 128:(c + 1) * 128],
                                     h1[k][:], start=(k == 0),
                                     stop=(k == F // 128 - 1))
                xx2 = wp.tile([128, T], f32, tag="xhT", name=f"x2_{c}")
                nc.vector.scalar_tensor_tensor(xx2[:], ps2[:], b2t[:, c:c + 1],
                                               x1[c][:], OP.add, OP.add)
                x2.append(xx2)

            # ---- rms2 + y (bf16) + label dot ----
            pssq = pp.tile([128, 512], f32, tag="pp")
            for c in range(NT):
                sq_s = scr()
                nc.scalar.activation(sq_s[:], x2[c][:], AF.Square)
                nc.tensor.matmul(pssq[0:1, :], ones128[:], sq_s[:],
                                 start=(c == 0), stop=(c == NT - 1))
            lr2 = wp.tile([1, T], f32, tag="row", bufs=2)
            nc.scalar.activation(lr2[:], pssq[0:1, :], AF.Ln)
            r2 = wp.tile([1, T], f32, tag="row", bufs=2)
            nc.scalar.activation(r2[:], lr2[:], AF.Exp, scale=-0.5,
                                 bias=lnsqd[0:1, 0:1])
            psrb = pp.tile([128, T], f32, tag="pp")
            nc.tensor.matmul(psrb[:], ones1x[:], r2[:], start=True, stop=True)

            # y per half + split AllGather (3a overlaps second half's compute)
            yhalf = [dp.tile([2 * 128, T], dt.bfloat16, name=f"yh{h}")
                     for h in range(2)]
            yall = [dp.tile([NCORES * 2 * 128, T], dt.bfloat16,
                            addr_space="Shared", name=f"ya{h}") for h in range(2)]
            pslab = pp.tile([128, 512], f32, tag="pp")
            for h, cs in ((0, H0), (1, H1)):
                for i, c in enumerate(cs):
                    t5 = scr()
                    nc.vector.tensor_mul(t5[:], x2[c][:], psrb[:])
                    yb = scr("scrb", (128, 512), dt.bfloat16, bufs=4)
                    nc.vector.tensor_scalar_mul(yb[:], t5[:], g2p[:, c:c + 1])
                    nc.sync.dma_start(out=yhalf[h][i * 128:(i + 1) * 128, :],
                                      in_=yb[:])
                    eb = scr("scrb", (128, 512), dt.bfloat16, bufs=4)
                    nc.vector.tensor_mul(eb[:], yb[:], wlabt[c][:])
                    nc.tensor.matmul(pslab[0:1, :], onesbf[:], eb[:],
                                     start=(c == 0), stop=(c == NT - 1))
                nc.gpsimd.collective_compute(
                    "AllGather", OP.bypass, replica_groups=[list(range(NCORES))],
                    ins=[yhalf[h].opt()], outs=[yall[h].opt()])
            labrow = wp.tile([1, T], f32, tag="row", bufs=2)
            nc.vector.tensor_copy(labrow[:], pslab[0:1, :])
            nc.sync.dma_start(out=labv_out[:], in_=labrow[:])

            yT = [bigp.tile([128, NTOK], dt.bfloat16, tag="w1y", name=f"yT{_k}")
                  for _k in range(NT)]
            for k in range(NT):
                h, i = (0, k) if k < 2 else (1, k - 2)
                for j in range(NCORES):
                    nc.sync.dma_start(
                        out=yT[k][:, j * T:(j + 1) * T],
                        in_=yall[h][j * 256 + i * 128: j * 256 + (i + 1) * 128, :])

            # ---- logits phase: vocab-sharded matmul + sum-exp ----
            # k-outer: the stationary yT m-tile is reused across all NSL
            # vocab slices before advancing K, amortizing weight loads.
            for m in range(MT):
                sem = wp.tile([128, NSL], f32, tag="sem", bufs=3,
                              name=f"sem{m}")
                psl_l = [pp.tile([128, T], f32, tag="pp", name=f"psl_{m}_{n}")
                         for n in range(NSL)]
                for k in range(NT):
                    for n in range(NSL):
                        nc.tensor.matmul(psl_l[n][:, 0:SL],
                                         yT[k][:, m * 128:(m + 1) * 128],
                                         wlgt[k][:, n * SL:(n + 1) * SL],
                                         start=(k == 0), stop=(k == NT - 1))
                for n in range(NSL):
                    lg = wp.tile([128, SL], f32, tag="lg", bufs=4,
                                 name=f"lg_{m}_{n}")
                    nc.vector.tensor_copy(lg[:], psl_l[n][:, 0:SL])
                    nc.sync.dma_start(
                        out=logits_out[m * 128:(m + 1) * 128, n * SL:(n + 1) * SL],
                        in_=lg[:])
                    exps = scr("exps", (128, SL), bufs=2)
                    nc.scalar.activation(exps[:], lg[:], AF.Exp,
                                         accum_out=sem[:, n:n + 1])
                nc.vector.tensor_reduce(sexp_all[:, m:m + 1], sem[:], AX.X, OP.add)

            nc.sync.dma_start(out=sexp_out[:], in_=sexp_all[:])

    _split_multi_waits(nc)
    return nc


_NC = None
LAST_EXEC_NS = None
LAST_RESULTS = None


def _get_nc():
    global _NC
    if _NC is None:
        _NC = _build()
    return _NC


def kernel(inputs, labels, emb, w_gru, gamma1, gamma2, w1, b1, w2, b2, w_logits):
    global LAST_EXEC_NS, LAST_RESULTS
    f32 = np.float32
    idx = np.asarray(inputs).reshape(-1).astype(np.int64)
    lab = np.asarray(labels).reshape(-1).astype(np.int64)
    emb = np.ascontiguousarray(np.asarray(emb, f32))
    w_gru = np.ascontiguousarray(np.asarray(w_gru, f32))
    w1 = np.ascontiguousarray(np.asarray(w1, f32))
    w2b = np.asarray(w2, f32).astype(bf16)
    wlgb = np.asarray(w_logits, f32).astype(bf16)
    wlabb = np.asarray(w_logits, f32)[:, lab].astype(bf16)  # [D, 4096]
    g1 = np.asarray(gamma1, f32); g2 = np.asarray(gamma2, f32)
    b1 = np.asarray(b1, f32); b2 = np.asarray(b2, f32)

    x_gather = emb[idx]  # [4096, 512]

    in_maps = []
    for j in range(NCORES):
        mask = np.zeros((128, 8), f32)
        if j % 4 != 0:
            mask[:, j - 1] = 1.0
        selb = np.full((128, 1), 1.0 if j < 4 else 0.0, f32)
        in_maps.append({
            "x0": np.ascontiguousarray(x_gather[j * T:(j + 1) * T]),
            "g1": g1, "g2": g2,
            "wg": w_gru, "w1": w1, "b1": b1,
            "w2": np.ascontiguousarray(w2b), "b2": b2,
            "wlg": np.ascontiguousarray(wlgb[:, j * VS:(j + 1) * VS]),
            "wlab": np.ascontiguousarray(wlabb[:, j * T:(j + 1) * T]),
            "mask": mask, "selb": selb,
        })

    nc = _get_nc()
    trace = bool(os.environ.get("BASS_KERNEL_PROFILE"))
    res = run_bass_kernel_spmd(nc, in_maps, list(range(NCORES)), trace=trace)
    LAST_EXEC_NS = res.exec_time_ns
    LAST_RESULTS = res

    r = res.results
    logits = np.concatenate([r[j]["logits"] for j in range(NCORES)], axis=1)
    logits = logits.reshape(B, S, V)
    s_tot = np.zeros((128, MT), f32)
    for j in range(NCORES):
        s_tot += r[j]["sexp"]
    lse = np.log(s_tot.T.reshape(-1))          # [4096], token order m*128+p
    labv = np.concatenate([r[j]["labv"][0] for j in range(NCORES)])
    loss = f32(-(labv - lse).mean())
    nh = r[0]["nh"].reshape(B, 1, D).astype(f32)
    return loss, logits, nh


# revision 19
# speedup vs baseline: 1.1253x; 1.1253x over previous
"""Trainium2 Bass kernel: minGRU block (emb-gather -> rmsnorm -> minGRU scan ->
MLP -> rmsnorm -> logits + log_softmax loss) on 8 NeuronCores.

Sharding: trunk token-sharded (8 chunks of 512 tokens; the reference's
log-space scan is reproduced faithfully -- including XLA-CPU exp
flush-to-zero -- with chunked prefix state exchanged via small AllGathers);
logits vocab-sharded (4000 cols/core) after an AllGather of the normalized
activations. Collectives are channel-split so their latency hides behind
the other half's compute. Host does gather/shard/concat + tiny loss combine.
"""
import os
import sys

import numpy as np
import ml_dtypes

for _p in ("/opt/trn_rl_repo", "/root/.axon_site/_ro/trn_rl_repo"):
    if os.path.isdir(_p) and _p not in sys.path:
        sys.path.append(_p)

import concourse.bass as bass
import concourse.tile as tile
import concourse.mybir as mybir
from concourse.bass_utils import run_bass_kernel_spmd
from concourse.masks import make_identity
from concourse.vector_clock import ScopedClock

dt = mybir.dt
AF = mybir.ActivationFunctionType
OP = mybir.AluOpType
AX = mybir.AxisListType
bf16 = ml_dtypes.bfloat16

NCORES = 8
B, S, D, V, F = 2, 2048, 512, 32000, 2048
T = 512              # tokens per core chunk
NT = D // 128        # 4 channel tiles
VS = V // NCORES     # 4000 vocab cols per core
SL = 500             # vocab slice per psum tile
NSL = VS // SL       # 8
MT = (B * S) // 128  # 32 token m-tiles
NTOK = B * S

EXP_FLUSH = -87.3365445          # XLA-CPU exp flush-to-zero cutoff
LN_SQRT_D = float(np.log(np.sqrt(np.float64(D))))
ERF_P = 0.47047                  # Abramowitz-Stegun 7.1.25
ERF_A = (0.3480242, -0.0958798, 0.7478556)
SQ2I = float(1.0 / np.sqrt(2.0))
LN2_64 = float(64 * np.log(2.0))

# ---------------------------------------------------------------------------
# Workaround: this container's walrus rejects instructions carrying more than
# one semaphore wait. Split extra waits onto single-wait NoOps (same engine).
# ---------------------------------------------------------------------------
_split_ctr = [0]


def _patched_drain_and_barrier(self, tick_clock, wait_clock):
    probe = self.nc.sync.nop(nofuse=True, hint="drain_waits")
    wait_clock.add_sem_waits(probe.ins, ScopedClock({None: tick_clock.global_clock}))
    si = probe.ins.sync_info
    if si is not None and si.on_wait and len(si.on_wait) > 1:
        extra = list(si.on_wait[1:])
        del si.on_wait[1:]
        for w in extra:
            n2 = self.nc.sync.nop(nofuse=True, hint="drain_waits")
            n2.ins.sync_info = mybir.SyncInfo(on_wait=[w], on_update=[])
    self.nc.sync.drain()
    self.nc.all_engine_barrier()
    assert self.sems is not None
    popped = self.nc._tile_sem_poison_stack.pop()
    assert popped is self._sem_poison
    self.nc.clear_and_free_semaphores(list(self.sems.allocated().values()))
    self.nc.all_engine_barrier()


tile.TileContext._drain_and_barrier = _patched_drain_and_barrier


def _split_multi_waits(nc):
    for fn in nc.m.functions:
        for bb in fn.blocks:
            new_insts = []
            for inst in bb.instructions:
                si = inst.sync_info
                if si is not None and si.on_wait and len(si.on_wait) > 1:
                    extra = list(si.on_wait[:-1])
                    keep = si.on_wait[-1]
                    for w in extra:
                        _split_ctr[0] += 1
                        new_insts.append(mybir.InstNoOp(
                            name=f"I-swsplit-{_split_ctr[0]}",
                            engine=inst.engine,
                            sync_info=mybir.SyncInfo(on_wait=[w], on_update=[]),
                            text_hint="split_wait",
                            bass_nofuse=True,
                        ))
                    si.on_wait = [keep]
                new_insts.append(inst)
            bb.instructions[:] = new_insts


# ---------------------------------------------------------------------------
# Device program
# ---------------------------------------------------------------------------
def _build():
    nc = bass.Bass("TRN2", target_bir_lowering=False, debug=False,
                   num_devices=NCORES)
    f32 = dt.float32

    x0_in = nc.dram_tensor("x0", [T, D], f32, kind="ExternalInput").ap()
    g1_in = nc.dram_tensor("g1", [D], f32, kind="ExternalInput").ap()
    g2_in = nc.dram_tensor("g2", [D], f32, kind="ExternalInput").ap()
    wg_in = nc.dram_tensor("wg", [D, 2 * D], f32, kind="ExternalInput").ap()
    w1_in = nc.dram_tensor("w1", [D, F], dt.float32r, kind="ExternalInput").ap()
    b1_in = nc.dram_tensor("b1", [F], f32, kind="ExternalInput").ap()
    w2_in = nc.dram_tensor("w2", [F, D], dt.bfloat16, kind="ExternalInput").ap()
    b2_in = nc.dram_tensor("b2", [D], f32, kind="ExternalInput").ap()
    wlg_in = nc.dram_tensor("wlg", [D, VS], dt.bfloat16, kind="ExternalInput").ap()
    wlab_in = nc.dram_tensor("wlab", [D, T], dt.bfloat16, kind="ExternalInput").ap()
    mask_in = nc.dram_tensor("mask", [128, 8], f32, kind="ExternalInput").ap()
    selb_in = nc.dram_tensor("selb", [128, 1], f32, kind="ExternalInput").ap()

    logits_out = nc.dram_tensor("logits", [NTOK, VS], f32, kind="ExternalOutput").ap()
    labv_out = nc.dram_tensor("labv", [1, T], f32, kind="ExternalOutput").ap()
    sexp_out = nc.dram_tensor("sexp", [128, MT], f32, kind="ExternalOutput").ap()
    nh_out = nc.dram_tensor("nh", [2, D], f32, kind="ExternalOutput").ap()

    with tile.TileContext(nc) as tc:
        with tc.tile_pool(name="const", bufs=1) as cp, \
             tc.tile_pool(name="io", bufs=1) as io, \
             tc.tile_pool(name="big", bufs=4) as bigp, \
             tc.tile_pool(name="work", bufs=4) as wp, \
             tc.tile_pool(name="ps", bufs=8, space="PSUM") as pp, \
             tc.tile_pool(name="dram", bufs=1, space="DRAM") as dp:

            # ---- warm-up collective: absorbs cross-core launch skew while
            # input DMAs stream (first collective ~60us, later ones ~13us) ----
            warm_b = dp.tile([1, 128], f32)
            warm_g = dp.tile([NCORES, 128], f32, addr_space="Shared")
            nc.gpsimd.collective_compute(
                "AllGather", OP.bypass, replica_groups=[list(range(NCORES))],
                ins=[warm_b.opt()], outs=[warm_g.opt()])

            # ---- constants ----
            ident = cp.tile([128, 128], f32, tag="ident")
            make_identity(nc, ident[:])
            zeros = cp.tile([128, 512], f32, tag="zeros")
            nc.vector.memset(zeros[:], 0.0)
            ones128 = cp.tile([128, 1], f32, tag="ones128")
            nc.vector.memset(ones128[:], 1.0)
            ones1x = cp.tile([1, 128], f32, tag="ones1x")
            nc.vector.memset(ones1x[:], 1.0)
            onesbf = cp.tile([128, 1], dt.bfloat16, tag="onesbf")
            nc.vector.memset(onesbf[:], 1.0)
            lnsqd = cp.tile([128, 1], f32, tag="lnsqd")
            nc.vector.memset(lnsqd[:], LN_SQRT_D)

            g1p = cp.tile([128, NT], f32, tag="g1p")
            nc.sync.dma_start(out=g1p[:], in_=g1_in.rearrange("(c p) -> p c", p=128))
            nc.vector.tensor_scalar_add(g1p[:], g1p[:], 1.0)
            g2p = cp.tile([128, NT], f32, tag="g2p")
            nc.sync.dma_start(out=g2p[:], in_=g2_in.rearrange("(c p) -> p c", p=128))
            nc.vector.tensor_scalar_add(g2p[:], g2p[:], 1.0)
            b1t = cp.tile([128, F // 128], f32, tag="b1t")
            nc.sync.dma_start(out=b1t[:], in_=b1_in.rearrange("(m p) -> p m", p=128))
            b1s = cp.tile([128, F // 128], f32, tag="b1s")
            nc.vector.tensor_scalar_mul(b1s[:], b1t[:], SQ2I)
            b2t = cp.tile([128, NT], f32, tag="b2t")
            nc.sync.dma_start(out=b2t[:], in_=b2_in.rearrange("(c p) -> p c", p=128))
            maskt = cp.tile([128, 8], f32, tag="maskt")
            nc.sync.dma_start(out=maskt[:], in_=mask_in[:])
            selb = cp.tile([128, 1], f32, tag="selb")
            nc.sync.dma_start(out=selb[:], in_=selb_in[:])
            sexp_all = cp.tile([128, MT], f32, tag="sexp_all")

            _scr_n = [0]

            def scr(tag="scr", shape=(128, 512), dtype=f32, bufs=5):
                _scr_n[0] += 1
                return wp.tile(list(shape), dtype, tag=tag, bufs=bufs,
                               name=f"{tag}_{_scr_n[0]}")

            # ---- stage 1: rms1 + transpose -> xhT (feature-major) ----
            xh0 = []
            for t in range(NT):
                x0t = scr("x0t", bufs=2)
                nc.sync.dma_start(out=x0t[:], in_=x0_in[t * 128:(t + 1) * 128, :])
                ssq = scr("t1", (128, 1), bufs=16)
                sq_s = scr()
                nc.scalar.activation(sq_s[:], x0t[:], AF.Square, accum_out=ssq[:])
                lssq = scr("t1", (128, 1), bufs=16)
                nc.scalar.activation(lssq[:], ssq[:], AF.Ln)
                r = scr("t1", (128, 1), bufs=16)
                nc.scalar.activation(r[:], lssq[:], AF.Exp, scale=-0.5,
                                     bias=lnsqd[:, 0:1])
                xh = scr("xh0", bufs=4)
                nc.vector.tensor_scalar_mul(xh[:], x0t[:], r[:, 0:1])
                xh0.append(xh)

            wgt = [io.tile([128, 2 * D], f32, tag=f"wgt{k}", name=f"wgt{k}")
                   for k in range(NT)]
            for k in range(NT):
                nc.sync.dma_start(out=wgt[k][:], in_=wg_in[k * 128:(k + 1) * 128, :])

            xhT = []
            for c in range(NT):
                psx = pp.tile([128, 512], f32, tag="pp")
                for t in range(NT):
                    nc.tensor.transpose(psx[:, t * 128:(t + 1) * 128],
                                        xh0[t][:, c * 128:(c + 1) * 128], ident[:])
                xt_ = wp.tile([128, T], f32, tag="xhT")
                nc.vector.tensor_scalar_mul(xt_[:], psx[:], g1p[:, c:c + 1])
                xhT.append(xt_)

            # ---- remaining weight loads (stream during trunk compute) ----
            w1t = [bigp.tile([128, F], dt.float32r, tag="w1y", name=f"w1t{_k}")
                   for _k in range(NT)]
            for k in range(NT):
                nc.sync.dma_start(out=w1t[k][:], in_=w1_in[k * 128:(k + 1) * 128, :])
            w2t = [io.tile([128, D], dt.bfloat16, tag=f"w2t{k}", name=f"w2t{k}")
                   for k in range(F // 128)]
            for k in range(F // 128):
                nc.sync.dma_start(out=w2t[k][:], in_=w2_in[k * 128:(k + 1) * 128, :])
            wlabt = [io.tile([128, T], dt.bfloat16, tag=f"wlab{c}", name=f"wlab{c}")
                     for c in range(NT)]
            for c in range(NT):
                nc.sync.dma_start(out=wlabt[c][:], in_=wlab_in[c * 128:(c + 1) * 128, :])
            wlgt = [io.tile([128, VS], dt.bfloat16, tag=f"wlg{k}", name=f"wlg{k}")
                    for k in range(NT)]
            for k in range(NT):
                nc.sync.dma_start(out=wlgt[k][:], in_=wlg_in[k * 128:(k + 1) * 128, :])

            # ---- stage 2: hg matmuls (fp32) + local log-space scan ----
            las = [None] * NT
            nloc = [None] * NT
            mnl_l = [None] * NT
            Sl = [None] * NT
            cumA_l = [None] * NT
            obias_l = [None] * NT
            ubias_l = [None] * NT
            M2b_l = [None] * NT
            x1 = [None] * NT
            cumU_l = [None] * NT

            def emit_nh():
                # next_hidden from gathered summaries (cols 3 and 7);
                # runs off the critical path, after the y collectives issue
                for c in range(NT):
                    cumU = cumU_l[c]
                    for bi, col in ((0, 3), (1, 7)):
                        cu3 = scr("t1", (128, 1), bufs=16)
                        nc.vector.tensor_copy(cu3[:], cumU[:, col:col + 1])
                        lnu = scr("t1", (128, 1), bufs=16)
                        nc.scalar.activation(lnu[:], cu3[:], AF.Ln)
                        t6 = scr("t1", (128, 1), bufs=16)
                        nc.vector.tensor_add(t6[:], cumA_l[c][:, col:col + 1], lnu[:])
                        nc.vector.tensor_sub(t6[:], t6[:], M2b_l[c][bi][:])
                        nc.vector.tensor_scalar_max(t6[:], t6[:], -100.0)
                        nhv = scr("t1", (128, 1), bufs=16)
                        nc.scalar.activation(nhv[:], t6[:], AF.Exp)
                        zm1 = scr("zm1", (128, 1), dt.uint8, bufs=4)
                        nc.vector.tensor_scalar(zm1[:], cu3[:], 0.0, None, OP.is_equal)
                        nc.vector.copy_predicated(nhv[:], zm1[:], zeros[:, 0:1])
                        nc.sync.dma_start(
                            out=nh_out[bi:bi + 1, c * 128:(c + 1) * 128].rearrange(
                                "o p -> p o"),
                            in_=nhv[:, 0:1])

            def phaseA(c):
                psh = pp.tile([128, T], f32, tag="pp", name=f"psh{c}")
                for k in range(NT):
                    nc.tensor.matmul(psh[:], wgt[k][:, c * 128:(c + 1) * 128],
                                     xhT[k][:], start=(k == 0), stop=(k == NT - 1))
                psg = pp.tile([128, T], f32, tag="pp", name=f"psg{c}")
                for k in range(NT):
                    nc.tensor.matmul(psg[:], wgt[k][:, (NT + c) * 128:(NT + c + 1) * 128],
                                     xhT[k][:], start=(k == 0), stop=(k == NT - 1))
                # gate: lc = -softplus(gate); lz = ln(1+e^-gate) (= -log_z)
                epg = scr()
                nc.scalar.activation(epg[:], psg[:], AF.Exp)
                nc.vector.tensor_scalar_add(epg[:], epg[:], 1.0)
                splus = scr()
                nc.scalar.activation(splus[:], epg[:], AF.Ln)
                lc = scr("lc", bufs=2)
                nc.vector.tensor_scalar_mul(lc[:], splus[:], -1.0)
                eng = scr()
                nc.scalar.activation(eng[:], psg[:], AF.Exp, scale=-1.0)
                nc.vector.tensor_scalar_add(eng[:], eng[:], 1.0)
                lz = scr()
                nc.scalar.activation(lz[:], eng[:], AF.Ln)
                # hidden: lg = h>=0 ? ln(h+0.5) : -ln(1+e^-h)
                hp5 = scr()
                nc.vector.tensor_scalar(hp5[:], psh[:], 0.5, 0.5, OP.add, OP.max)
                lgb1 = scr()
                nc.scalar.activation(lgb1[:], hp5[:], AF.Ln)
                enh = scr()
                nc.scalar.activation(enh[:], psh[:], AF.Exp, scale=-1.0)
                nc.vector.tensor_scalar_add(enh[:], enh[:], 1.0)
                lgb2 = scr()
                nc.scalar.activation(lgb2[:], enh[:], AF.Ln)
                nc.vector.tensor_scalar_mul(lgb2[:], lgb2[:], -1.0)
                mneg = scr("mneg", (128, 512), dt.uint8, bufs=2)
                nc.vector.tensor_scalar(mneg[:], psh[:], 0.0, None, OP.is_lt)
                nc.vector.copy_predicated(lgb1[:], mneg[:], lgb2[:])
                lv = scr()
                nc.vector.tensor_sub(lv[:], lgb1[:], lz[:])
                la = wp.tile([128, T], f32, tag="las", name=f"las{c}")
                nc.vector.tensor_tensor_scan(la[:], lc[:], zeros[:], 0.0,
                                             OP.add, OP.add)
                nl = wp.tile([128, T], f32, tag="nloc", name=f"nloc{c}")
                nc.vector.tensor_sub(nl[:], la[:], lv[:])
                mn = scr("t1", (128, 1), bufs=16)
                nc.vector.tensor_reduce(mn[:], nl[:], AX.X, OP.min)
                las[c] = la; nloc[c] = nl; mnl_l[c] = mn

            def coll1(h, cs):
                w_ = 128 * len(cs)
                sm = dp.tile([2, w_], f32, name=f"sm1_{h}")
                for i, c in enumerate(cs):
                    nc.sync.dma_start(
                        out=sm[0:1, i * 128:(i + 1) * 128].rearrange("o p -> p o"),
                        in_=las[c][:, T - 1:T])
                    nc.sync.dma_start(
                        out=sm[1:2, i * 128:(i + 1) * 128].rearrange("o p -> p o"),
                        in_=mnl_l[c][:, 0:1])
                gth = dp.tile([2 * NCORES, w_], f32, addr_space="Shared",
                              name=f"gth1_{h}")
                nc.gpsimd.collective_compute(
                    "AllGather", OP.bypass, replica_groups=[list(range(NCORES))],
                    ins=[sm.opt()], outs=[gth.opt()])
                return gth[:, :].rearrange("(j r) (c p) -> r c p j", r=2, c=len(cs))

            def post1(c, g1v, i):
                totAg = scr("t8", (128, 8), bufs=16)
                nc.sync.dma_start(out=totAg[:], in_=g1v[0, i])
                mng = scr("t8", (128, 8), bufs=16)
                nc.sync.dma_start(out=mng[:], in_=g1v[1, i])
                cumA = wp.tile([128, 8], f32, tag="cAi", name=f"cAi{c}")
                for b in range(2):
                    nc.vector.tensor_tensor_scan(
                        cumA[:, b * 4:(b + 1) * 4], totAg[:, b * 4:(b + 1) * 4],
                        zeros[:, 0:4], 0.0, OP.add, OP.add)
                cumAx = scr("t8", (128, 8), bufs=16)
                nc.vector.tensor_sub(cumAx[:], cumA[:], totAg[:])
                ng = scr("t8", (128, 8), bufs=16)
                nc.vector.tensor_add(ng[:], mng[:], cumAx[:])
                mnb0 = wp.tile([128, 1], f32, tag="mnb0", name=f"mnb0_{c}")
                nc.vector.tensor_reduce(mnb0[:], ng[:, 0:4], AX.X, OP.min)
                mnb1 = wp.tile([128, 1], f32, tag="mnb1", name=f"mnb1_{c}")
                nc.vector.tensor_reduce(mnb1[:], ng[:, 4:8], AX.X, OP.min)
                dmn = scr("t1", (128, 1), bufs=16)
                nc.vector.tensor_sub(dmn[:], mnb0[:], mnb1[:])
                M2 = scr("t1", (128, 1), bufs=16)
                nc.vector.scalar_tensor_tensor(M2[:], dmn[:], selb[:, 0:1],
                                               mnb1[:], OP.mult, OP.add)
                tm8 = scr("t8", (128, 8), bufs=16)
                nc.vector.tensor_mul(tm8[:], cumA[:], maskt[:])
                Aprev = scr("t1", (128, 1), bufs=16)
                nc.vector.tensor_reduce(Aprev[:], tm8[:], AX.X, OP.add)
                ub = wp.tile([128, 1], f32, tag="ubias", name=f"ub{c}")
                nc.vector.tensor_sub(ub[:], M2[:], Aprev[:])
                ob = wp.tile([128, 1], f32, tag="obias", name=f"ob{c}")
                nc.vector.tensor_sub(ob[:], Aprev[:], M2[:])
                cumA_l[c] = cumA; obias_l[c] = ob
                ubias_l[c] = ub; M2b_l[c] = (mnb0, mnb1)

            def phaseB(c):
                arg = scr()
                nc.vector.tensor_scalar(arg[:], nloc[c][:], -1.0,
                                        ubias_l[c][:, 0:1], OP.mult, OP.add)
                u = scr()
                nc.scalar.activation(u[:], arg[:], AF.Exp)
                um = scr("mneg", (128, 512), dt.uint8, bufs=2)
                nc.vector.tensor_scalar(um[:], arg[:], EXP_FLUSH, None, OP.is_lt)
                nc.vector.copy_predicated(u[:], um[:], zeros[:])
                Sc = wp.tile([128, T], f32, tag="Sl", name=f"Sl{c}")
                nc.vector.tensor_tensor_scan(Sc[:], u[:], zeros[:], 0.0,
                                             OP.add, OP.add)
                Sl[c] = Sc

            def coll2(h, cs):
                w_ = 128 * len(cs)
                sm = dp.tile([1, w_], f32, name=f"sm2_{h}")
                for i, c in enumerate(cs):
                    nc.sync.dma_start(
                        out=sm[0:1, i * 128:(i + 1) * 128].rearrange("o p -> p o"),
                        in_=Sl[c][:, T - 1:T])
                gth = dp.tile([NCORES, w_], f32, addr_space="Shared",
                              name=f"gth2_{h}")
                nc.gpsimd.collective_compute(
                    "AllGather", OP.bypass, replica_groups=[list(range(NCORES))],
                    ins=[sm.opt()], outs=[gth.opt()])
                return gth[:, :].rearrange("j (c p) -> c p j", c=len(cs))

            def phaseC(c, g2v, i):
                usg = scr("t8", (128, 8), bufs=16)
                nc.sync.dma_start(out=usg[:], in_=g2v[i])
                cumU = wp.tile([128, 8], f32, tag="cUi", name=f"cUi{c}")
                for b in range(2):
                    nc.vector.tensor_tensor_scan(
                        cumU[:, b * 4:(b + 1) * 4], usg[:, b * 4:(b + 1) * 4],
                        zeros[:, 0:4], 0.0, OP.add, OP.add)
                cumUx = scr("t8", (128, 8), bufs=16)
                nc.vector.tensor_sub(cumUx[:], cumU[:], usg[:])
                tm8 = scr("t8", (128, 8), bufs=16)
                nc.vector.tensor_mul(tm8[:], cumUx[:], maskt[:])
                Sprev = scr("t1", (128, 1), bufs=16)
                nc.vector.tensor_reduce(Sprev[:], tm8[:], AX.X, OP.add)
                Sfull = scr()
                nc.vector.tensor_scalar_add(Sfull[:], Sl[c][:], Sprev[:, 0:1])
                lnS = scr()
                nc.scalar.activation(lnS[:], Sfull[:], AF.Ln)
                # ACT Ln clamps below ~2e-19: rescale small S by 2^64
                Ss = scr()
                nc.vector.tensor_scalar_mul(Ss[:], Sfull[:], float(2.0 ** 64))
                lnB = scr()
                nc.scalar.activation(lnB[:], Ss[:], AF.Ln)
                nc.vector.tensor_scalar_add(lnB[:], lnB[:], -LN2_64)
                msm = scr("mneg", (128, 512), dt.uint8, bufs=2)
                nc.vector.tensor_scalar(msm[:], Sfull[:], 1e-15, None, OP.is_lt)
                nc.vector.copy_predicated(lnS[:], msm[:], lnB[:])
                zm = scr("mneg", (128, 512), dt.uint8, bufs=2)
                nc.vector.tensor_scalar(zm[:], Sfull[:], 0.0, None, OP.is_equal)
                ex = scr()
                nc.vector.tensor_add(ex[:], las[c][:], lnS[:])
                nc.vector.tensor_scalar_add(ex[:], ex[:], obias_l[c][:, 0:1])
                gru = scr()
                nc.scalar.activation(gru[:], ex[:], AF.Exp)
                nc.vector.copy_predicated(gru[:], zm[:], zeros[:])
                xx = wp.tile([128, T], f32, tag="las", name=f"x1_{c}")
                nc.vector.tensor_add(xx[:], gru[:], xhT[c][:])
                x1[c] = xx
                cumU_l[c] = cumU

            # channel-split pipeline: each collective's latency hides behind
            # the other half's compute
            H0, H1 = [0, 1], [2, 3]
            ALL4 = [0, 1, 2, 3]
            phaseA(0); phaseA(1); phaseA(2); phaseA(3)
            g1 = coll1(0, ALL4)
            for c in ALL4:
                post1(c, g1, c)
                phaseB(c)
            g2 = coll2(0, ALL4)
            for c in ALL4:
                phaseC(c, g2, c)

            # ---- stage 3: MLP (w1 f32r, gelu via erf poly, w2 bf16) ----
            x1r = []
            for c in range(NT):
                xr = wp.tile([128, T], dt.float32r, tag="nloc", name=f"x1r{c}")
                nc.vector.tensor_copy(xr[:], x1[c][:])
                x1r.append(xr)

            h1 = []
            for m in range(F // 128):
                ps1 = pp.tile([128, T], f32, tag="pp", name=f"ps1_{m}")
                for k in range(NT):
                    nc.tensor.matmul(ps1[:], w1t[k][:, m * 128:(m + 1) * 128],
                                     x1r[k][:], start=(k == 0), stop=(k == NT - 1))
                # gelu = 0.5*xg + 0.5*|xg|*(1-q), q = erf-poly(|xg|/sqrt2)*e^-t^2
                # poly intermediates in bf16 (DVE 2x/4x modes); 0.5*xg stays fp32
                tt = scr("scb", (128, 512), dt.bfloat16, bufs=8)
                nc.scalar.activation(tt[:], ps1[:], AF.Abs, scale=SQ2I,
                                     bias=b1s[:, m:m + 1])
                ax = scr("scb", (128, 512), dt.bfloat16, bufs=8)
                nc.scalar.activation(ax[:], ps1[:], AF.Abs, bias=b1t[:, m:m + 1])
                wv = scr("scb", (128, 512), dt.bfloat16, bufs=8)
                nc.vector.tensor_scalar(wv[:], tt[:], ERF_P, 1.0, OP.mult, OP.add)
                lnw = scr("scb", (128, 512), dt.bfloat16, bufs=8)
                nc.scalar.activation(lnw[:], wv[:], AF.Ln)
                kv = scr("scb", (128, 512), dt.bfloat16, bufs=8)
                nc.scalar.activation(kv[:], lnw[:], AF.Exp, scale=-1.0)
                a1 = scr("scb", (128, 512), dt.bfloat16, bufs=8)
                nc.vector.tensor_scalar_mul(a1[:], kv[:], ERF_A[2])
                a2 = scr("scb", (128, 512), dt.bfloat16, bufs=8)
                nc.vector.scalar_tensor_tensor(a2[:], a1[:], ERF_A[1], kv[:],
                                               OP.add, OP.mult)
                a3 = scr("scb", (128, 512), dt.bfloat16, bufs=8)
                nc.vector.scalar_tensor_tensor(a3[:], a2[:], ERF_A[0], kv[:],
                                               OP.add, OP.mult)
                sq2 = scr("scb", (128, 512), dt.bfloat16, bufs=8)
                nc.scalar.activation(sq2[:], tt[:], AF.Square)
                ee = scr("scb", (128, 512), dt.bfloat16, bufs=8)
                nc.scalar.activation(ee[:], sq2[:], AF.Exp, scale=-1.0)
                qv = scr("scb", (128, 512), dt.bfloat16, bufs=8)
                nc.vector.tensor_mul(qv[:], a3[:], ee[:])
                nq = scr("scb", (128, 512), dt.bfloat16, bufs=8)
                nc.vector.tensor_scalar(nq[:], qv[:], -0.5, 0.5, OP.mult, OP.add)
                r3 = scr("scb", (128, 512), dt.bfloat16, bufs=8)
                nc.vector.tensor_mul(r3[:], ax[:], nq[:])
                hx = scr()
                nc.vector.tensor_scalar(hx[:], ps1[:], b1t[:, m:m + 1], 0.5,
                                        OP.add, OP.mult)
                hm = wp.tile([128, T], dt.bfloat16, tag="h1", bufs=16,
                             name=f"h1_{m}")
                nc.vector.tensor_add(hm[:], r3[:], hx[:])
                h1.append(hm)

            x2 = []
            for c in range(NT):
                ps2 = pp.tile([128, T], f32, tag="pp", name=f"ps2_{c}")
                for k in range(F // 128):
                    nc.tensor.matmul(ps2[:], w2t[k][:, c * 128:(c + 1) * 128],
                                     h1[k][:], start=(k == 0),
                                     stop=(k == F // 128 - 1))
                xx2 = wp.tile([128, T], f32, tag="xhT", name=f"x2_{c}")
                nc.vector.scalar_tensor_tensor(xx2[:], ps2[:], b2t[:, c:c + 1],
                                               x1[c][:], OP.add, OP.add)
                x2.append(xx2)

            # ---- rms2 + y (bf16) + label dot ----
            pssq = pp.tile([128, 512], f32, tag="pp")
            for c in range(NT):
                sq_s = scr()
                nc.scalar.activation(sq_s[:], x2[c][:], AF.Square)
                nc.tensor.matmul(pssq[0:1, :], ones128[:], sq_s[:],
                                 start=(c == 0), stop=(c == NT - 1))
            lr2 = wp.tile([1, T], f32, tag="row", bufs=2)
            nc.scalar.activation(lr2[:], pssq[0:1, :], AF.Ln)
            r2 = wp.tile([1, T], f32, tag="row", bufs=2)
            nc.scalar.activation(r2[:], lr2[:], AF.Exp, scale=-0.5,
                                 bias=lnsqd[0:1, 0:1])
            psrb = pp.tile([128, T], f32, tag="pp")
            nc.tensor.matmul(psrb[:], ones1x[:], r2[:], start=True, stop=True)

            # y per half + split AllGather (3a overlaps second half's compute)
            yhalf = [dp.tile([2 * 128, T], dt.bfloat16, name=f"yh{h}")
                     for h in range(2)]
            yall = [dp.tile([NCORES * 2 * 128, T], dt.bfloat16,
                            addr_space="Shared", name=f"ya{h}") for h in range(2)]
            pslab = pp.tile([128, 512], f32, tag="pp")
            for h, cs in ((0, H0), (1, H1)):
                for i, c in enumerate(cs):
                    t5 = scr()
                    nc.vector.tensor_mul(t5[:], x2[c][:], psrb[:])
                    yb = scr("scrb", (128, 512), dt.bfloat16, bufs=4)
                    nc.vector.tensor_scalar_mul(yb[:], t5[:], g2p[:, c:c + 1])
                    nc.sync.dma_start(out=yhalf[h][i * 128:(i + 1) * 128, :],
                                      in_=yb[:])
                    eb = scr("scrb", (128, 512), dt.bfloat16, bufs=4)
                    nc.vector.tensor_mul(eb[:], yb[:], wlabt[c][:])
                    nc.tensor.matmul(pslab[0:1, :], onesbf[:], eb[:],
                                     start=(c == 0), stop=(c == NT - 1))
                nc.gpsimd.collective_compute(
                    "AllGather", OP.bypass, replica_groups=[list(range(NCORES))],
                    ins=[yhalf[h].opt()], outs=[yall[h].opt()])
            labrow = wp.tile([1, T], f32, tag="row", bufs=2)
            nc.vector.tensor_copy(labrow[:], pslab[0:1, :])
            nc.sync.dma_start(out=labv_out[:], in_=labrow[:])
            emit_nh()

            yT = [bigp.tile([128, NTOK], dt.bfloat16, tag="w1y", name=f"yT{_k}")
                  for _k in range(NT)]
            for k in range(NT):
                h, i = (0, k) if k < 2 else (1, k - 2)
                for j in range(NCORES):
                    nc.sync.dma_start(
                        out=yT[k][:, j * T:(j + 1) * T],
                        in_=yall[h][j * 256 + i * 128: j * 256 + (i + 1) * 128, :])

            # ---- logits phase: vocab-sharded matmul + sum-exp ----
            # k-outer: the stationary yT m-tile is reused across all NSL
            # vocab slices before advancing K, amortizing weight loads.
            for m in range(MT):
                sem = wp.tile([128, NSL], f32, tag="sem", bufs=3,
                              name=f"sem{m}")
                psl_l = [pp.tile([128, T], f32, tag="pp", name=f"psl_{m}_{n}")
                         for n in range(NSL)]
                for k in range(NT):
                    for n in range(NSL):
                        nc.tensor.matmul(psl_l[n][:, 0:SL],
                                         yT[k][:, m * 128:(m + 1) * 128],
                                         wlgt[k][:, n * SL:(n + 1) * SL],
                                         start=(k == 0), stop=(k == NT - 1))
                for n in range(NSL):
                    lg = wp.tile([128, SL], f32, tag="lg", bufs=4,
                                 name=f"lg_{m}_{n}")
                    nc.vector.tensor_copy(lg[:], psl_l[n][:, 0:SL])
                    nc.sync.dma_start(
                        out=logits_out[m * 128:(m + 1) * 128, n * SL:(n + 1) * SL],
                        in_=lg[:])
                    exps = scr("exps", (128, SL), bufs=2)
                    nc.scalar.activation(exps[:], lg[:], AF.Exp,
                                         accum_out=sem[:, n:n + 1])
                nc.vector.tensor_reduce(sexp_all[:, m:m + 1], sem[:], AX.X, OP.add)

            nc.sync.dma_start(out=sexp_out[:], in_=sexp_all[:])

    _split_multi_waits(nc)
    return nc


_NC = None
LAST_EXEC_NS = None
LAST_RESULTS = None


def _get_nc():
    global _NC
    if _NC is None:
        _NC = _build()
    return _NC


def kernel(inputs, labels, emb, w_gru, gamma1, gamma2, w1, b1, w2, b2, w_logits):
    global LAST_EXEC_NS, LAST_RESULTS
    f32 = np.float32
    idx = np.asarray(inputs).reshape(-1).astype(np.int64)
    lab = np.asarray(labels).reshape(-1).astype(np.int64)
    emb = np.ascontiguousarray(np.asarray(emb, f32))
    w_gru = np.ascontiguousarray(np.asarray(w_gru, f32))
    w1 = np.ascontiguousarray(np.asarray(w1, f32))
    w2b = np.asarray(w2, f32).astype(bf16)
    wlgb = np.asarray(w_logits, f32).astype(bf16)
    wlabb = np.asarray(w_logits, f32)[:, lab].astype(bf16)  # [D, 4096]
    g1 = np.asarray(gamma1, f32); g2 = np.asarray(gamma2, f32)
    b1 = np.asarray(b1, f32); b2 = np.asarray(b2, f32)

    x_gather = emb[idx]  # [4096, 512]

    in_maps = []
    for j in range(NCORES):
        mask = np.zeros((128, 8), f32)
        if j % 4 != 0:
            mask[:, j - 1] = 1.0
        selb = np.full((128, 1), 1.0 if j < 4 else 0.0, f32)
        in_maps.append({
            "x0": np.ascontiguousarray(x_gather[j * T:(j + 1) * T]),
            "g1": g1, "g2": g2,
            "wg": w_gru, "w1": w1, "b1": b1,
            "w2": np.ascontiguousarray(w2b), "b2": b2,
            "wlg": np.ascontiguousarray(wlgb[:, j * VS:(j + 1) * VS]),
            "wlab": np.ascontiguousarray(wlabb[:, j * T:(j + 1) * T]),
            "mask": mask, "selb": selb,
        })

    nc = _get_nc()
    trace = bool(os.environ.get("BASS_KERNEL_PROFILE"))
    res = run_bass_kernel_spmd(nc, in_maps, list(range(NCORES)), trace=trace)
    LAST_EXEC_NS = res.exec_time_ns
    LAST_RESULTS = res

    r = res.results
    logits = np.concatenate([r[j]["logits"] for j in range(NCORES)], axis=1)
    logits = logits.reshape(B, S, V)
    s_tot = np.zeros((128, MT), f32)
    for j in range(NCORES):
        s_tot += r[j]["sexp"]
    lse = np.log(s_tot.T.reshape(-1))          # [4096], token order m*128+p
    labv = np.concatenate([r[j]["labv"][0] for j in range(NCORES)])
    loss = f32(-(labv - lse).mean())
    nh = r[0]["nh"].reshape(B, 1, D).astype(f32)
    return loss, logits, nh
